# revision 9
# baseline (speedup 1.0000x reference)
"""CondensedLinearFineGrained on 8 TRN2 NeuronCores.

Math: out[b,o] = sum_k W[o,k] * input[b, mask[o,k]] + bias[o]
with B=256, IN_F=4096, OUT_F=4096, K=256.

Strategy
--------
Reformulate as a dense matmul:  out = input @ A^T + bias  where
A[o,f] = sum_{k: mask[o,k]==f} W[o,k]  (duplicates within a row are summed).

Sharding: output neurons, 512 per core. Per core:
  - input^T arrives as bf16 f-major tiles [128f x 32t x 256b] (2MB)
  - The first N_DENSE_TILES A^T f-tiles [128f x 512o] bf16 are shipped
    pre-densified from the host; the rest are built on-device by gpsimd
    local_scatter from host-repacked CSC (per-feature (o,weight) lists,
    deduped, -1-padded, int16 indices), two tiles per call.
  - TensorE accumulates psum[128b x 512o] over the 32 f-tiles; PSUM is
    seeded with bias via a K=1 bf16 matmul (ones^T @ bias broadcasts it
    across partitions).

Schedule (all timings vs the ~35.8us baseline this replaces):
  - PE issues N_WARM dependency-free dummy matmuls immediately after the
    engine preamble so the HAM clock gate reaches 2.4GHz before the real
    stream starts (was: warm MMs gated on a vector memset, HAM flipped
    5us INTO the real stream).
  - DMA chunks are small at the head (1-2 tiles) and grow, interleaved
    across the two HWDGE queues in PE-consumption order, so the first
    real matmul starts ~4us earlier.
  - Output is bf16 (error budget allows; host casts back to f32) and the
    last TAIL_SPLIT tiles run batch-half-0 first so psum0's copy/out-DMA
    overlap the remaining matmuls.
  - Semaphores are recycled INSIDE the single block (sync waits on
    done-sems then clears); the separate cleanup block cost ~4us of
    instruction-fetch barrier at the end.
"""

import numpy as np
import ml_dtypes

B = 256
IN_F = 4096
OUT_F = 4096
K = 256
N_CORES = 8
O_SH = OUT_F // N_CORES  # 512 output rows per core
NT = IN_F // 128         # 32 feature tiles
NB = B // 128            # 2 batch tiles

# f-tiles [0, N_DENSE_TILES) are DMA'd pre-densified; the rest are scattered
# on-device by gpsimd, two tiles per local_scatter. NT - N_DENSE_TILES must
# be even.
N_DENSE_TILES = 18
N_WARM = 12       # dependency-free dummy PE matmuls to release the HAM throttle
TAIL_SPLIT = 4    # trailing tiles run b0-first so psum0 finishes early
OUT_BF16 = True

# progressive chunk sizes (in f-tiles) for the two DMA streams. Chunks below
# ~256KB waste ring time on the per-transfer completion fence (~1.3us), so
# keep them chunky.
IN_CHUNKS = [4, 4, 8, 8, 8]
ATD_CHUNKS = [2, 2, 4, 4, 6]
# tiles at which an extra dummy matmul is inserted before the chunk waits:
# keeps the HAM clock gate warm across DMA-paced stalls in the dense phase
WARM_AT = (2, 4, 6, 8, 10, 12, 14, 16)

_BF16 = ml_dtypes.bfloat16

_prog_cache = {}


def _chunk_bounds(total, sizes):
    out, p, i = [], 0, 0
    while p < total:
        s = min(sizes[min(i, len(sizes) - 1)], total - p)
        out.append((p, p + s))
        p += s
        i += 1
    return out


def _build_program_raw(wpad: int, n_dense: int):
    """Hand-scheduled SPMD program: explicit per-engine streams + semaphores."""
    from contextlib import ExitStack
    from concourse import bacc, mybir, library_config

    nt_s = NT - n_dense
    npair = nt_s // 2
    assert nt_s % 2 == 0

    atd_chunks = _chunk_bounds(n_dense, ATD_CHUNKS)
    in_chunks = _chunk_bounds(NT, IN_CHUNKS)
    n_atd_ch = len(atd_chunks)
    n_in_ch = len(in_chunks)

    def chunk_of(chunks, t):
        for c, (c0, c1) in enumerate(chunks):
            if c0 <= t < c1:
                return c
        raise AssertionError

    nc = bacc.Bacc("TRN2", target_bir_lowering=False, debug=False)
    dt = mybir.dt
    out_dt = dt.bfloat16 if OUT_BF16 else dt.float32

    inT_d = nc.dram_tensor("inT", [128, NT, B], dt.bfloat16, kind="ExternalInput")
    bias_d = nc.dram_tensor("bias", [1, O_SH], dt.bfloat16, kind="ExternalInput")
    if npair:
        idx_d = nc.dram_tensor("cscidx", [128, npair, wpad], dt.int16,
                               kind="ExternalInput")
        val_d = nc.dram_tensor("cscval", [128, npair, wpad], dt.bfloat16,
                               kind="ExternalInput")
    if n_dense:
        atd_d = nc.dram_tensor("atd", [128, n_dense, O_SH], dt.bfloat16,
                               kind="ExternalInput")
    out_d = nc.dram_tensor("out", [NB, 128, O_SH], out_dt,
                           kind="ExternalOutput")

    inT_sb = nc.alloc_sbuf_tensor("inT_sb", [128, NT, B], dt.bfloat16).ap()
    bias_sb = nc.alloc_sbuf_tensor("bias_sb", [1, O_SH], dt.bfloat16).ap()
    ones_sb = nc.alloc_sbuf_tensor("ones_sb", [1, 128], dt.bfloat16).ap()
    # warm region is intentionally never initialized: the dummy matmuls only
    # exist to keep the PE busy, their values are irrelevant
    warm_sb = nc.alloc_sbuf_tensor("warm_sb", [128, 128 + O_SH],
                                   dt.bfloat16).ap()
    if npair:
        idx_sb = nc.alloc_sbuf_tensor("idx_sb", [128, npair, wpad],
                                      dt.int16).ap()
        val_sb = nc.alloc_sbuf_tensor("val_sb", [128, npair, wpad],
                                      dt.bfloat16).ap()
        at_sb = nc.alloc_sbuf_tensor("at_sb", [128, npair, 2, O_SH],
                                     dt.bfloat16).ap()
    if n_dense:
        atd_sb = nc.alloc_sbuf_tensor("atd_sb", [128, n_dense, O_SH],
                                      dt.bfloat16).ap()
    outs_sb = [nc.alloc_sbuf_tensor(f"out_sb{i}", [128, O_SH], out_dt).ap()
               for i in range(NB)]

    psums = [nc.alloc_psum_tensor(f"ps{i}", [128, O_SH], dt.float32).ap()
             for i in range(NB)]
    ps_warm = nc.alloc_psum_tensor("ps_warm", [128, O_SH], dt.float32).ap()

    with ExitStack() as ctx:
        sem = lambda name: ctx.enter_context(nc.semaphore(name))
        # One semaphore per DMA: sub-transfers of back-to-back DMAs on one
        # queue can complete out of order, so prefix thresholds on a shared
        # semaphore would be unsound.
        s_bias = sem("s_bias")
        s_ci = sem("s_ci") if npair else None
        s_cv = sem("s_cv") if npair else None
        s_in = [sem(f"s_in{c}") for c in range(n_in_ch)]
        s_atd = [sem(f"s_atd{c}") for c in range(n_atd_ch)]
        # out-DMA completion sems: incremented (the BIR verifier requires a
        # sem update on every DMA) but never waited or cleared — the
        # runtime's queue drain covers output completion.
        s_od = [sem(f"s_od{i}") for i in range(NB)]
        s_g = sem("s_g")    # scatter pairs published
        s_v = sem("s_v")    # DVE consts ready
        s_ps = sem("s_ps")  # PE accumulation done per psum
        s_cp = sem("s_cp")  # psum->sbuf copies done

        recycle_sems = ([s_bias] + ([s_ci, s_cv] if npair else [])
                        + s_in + s_atd + [s_g, s_v, s_ps, s_cp])

        # Interleave both input streams across the two HWDGE queues in
        # PE-consumption order (PE eats 2 bytes of dense A^T per byte of
        # input^T), greedily byte-balanced so neither queue starves the PE.
        # The CSC arrays ride the same queues (the gpsimd SWDGE path is
        # blocked ~6us by the local_scatter IRAM library load), placed so
        # they land by the time the library is ready (~13.4us).
        feed = []
        for c, (c0, c1) in enumerate(atd_chunks):
            feed.append((c0, 0, atd_sb[:, c0:c1, :], atd_d[:, c0:c1, :],
                         s_atd[c], (c1 - c0) * O_SH * 2))
        for c, (c0, c1) in enumerate(in_chunks):
            feed.append((c0, 1, inT_sb[:, c0:c1, :], inT_d[:, c0:c1, :],
                         s_in[c], (c1 - c0) * B * 2))
        if npair:
            # CSC leads both queues: the scatters can't start before the
            # ~6us local_scatter IRAM load finishes (~13.4us), and Pool
            # paces the whole back half if its input arrives later
            csc_b = 128 * npair * wpad * 2
            feed.append((-1, 2, idx_sb[:], idx_d[:], s_ci, csc_b))
            feed.append((-1, 3, val_sb[:], val_d[:], s_cv, csc_b))
        feed.sort(key=lambda f: (f[0], f[1]))
        qa, qb, ba, bb = [], [], 0, 0
        for _, _, dst, src, s, w in feed:
            if ba <= bb:
                qa.append((dst, src, s)); ba += w
            else:
                qb.append((dst, src, s)); bb += w

        with nc.Block() as block:

            @block.sync
            def _(sy):
                for dst, src, s in qa:
                    sy.dma_start(out=dst, in_=src).then_inc(s, 16)
                # out1 DMA (out0 goes on scalar); completion not waited —
                # the runtime queue drain covers it
                sy.wait_ge(s_cp, 2)
                sy.dma_start(out=out_d[1],
                             in_=outs_sb[1][:]).then_inc(s_od[1], 16)
                # recycle semaphores inline: all waiters are done once the
                # copies (s_cp) and scatters (s_g) have completed, so the
                # next execution of this NEFF starts from zero without a
                # separate cleanup block (which would pay its own
                # instruction-fetch barrier).
                if npair:
                    sy.wait_ge(s_g, npair)
                for s in recycle_sems:
                    sy.sem_clear(s)

            @block.scalar
            def _(sc):
                sc.dma_start(out=bias_sb[:], in_=bias_d[:]).then_inc(s_bias, 16)
                for dst, src, s in qb:
                    sc.dma_start(out=dst, in_=src).then_inc(s, 16)
                sc.wait_ge(s_cp, 1)
                sc.dma_start(out=out_d[0],
                             in_=outs_sb[0][:]).then_inc(s_od[0], 16)

            @block.vector
            def _(v):
                v.memset(ones_sb[:], 1.0)
                v.drain()
                v.sem_inc(s_v, 1)
                for i in range(NB):
                    v.wait_ge(s_ps, i + 1)
                    v.tensor_copy(outs_sb[i][:],
                                  psums[i][:]).then_inc(s_cp, 1)

            if npair:
                @block.gpsimd
                def _(g):
                    g.load_library(library_config.local_scatter)
                    g.wait_ge(s_ci, 16)
                    g.wait_ge(s_cv, 16)
                    for j in range(npair):
                        g.local_scatter(
                            at_sb[:, j],
                            val_sb[:, j],
                            idx_sb[:, j],
                            channels=128,
                            num_elems=2 * O_SH,
                            num_idxs=wpad,
                        ).then_inc(s_g, 1)

            @block.tensor
            def _(te):
                # dependency-free dummy matmuls from t~0: the HAM clock gate
                # needs ~3.4us of sustained PE activity to reach 2.4GHz, so
                # these run while the input DMAs are still in flight. The
                # warm region is uninitialized garbage; results go to a PSUM
                # bank that is never read.
                for _ in range(N_WARM):
                    te.matmul(ps_warm[:], warm_sb[:, :128], warm_sb[:, 128:],
                              start=True, stop=True, skip_group_check=True)
                te.wait_ge(s_v, 1)
                te.wait_ge(s_bias, 16)
                for i in range(NB):
                    te.matmul(psums[i][:], ones_sb[:], bias_sb[:],
                              start=True, stop=False)

                seen = set()
                g_thr = 0

                def wait_once(s):
                    if s.name not in seen:
                        te.wait_ge(s, 16)
                        seen.add(s.name)

                def rhs_of(t):
                    nonlocal g_thr
                    if t in WARM_AT:
                        # insurance against HAM re-throttle: a dummy matmul
                        # right before a chunk wait keeps the PE from sitting
                        # fully idle through a DMA-paced stall
                        te.matmul(ps_warm[:], warm_sb[:, :128],
                                  warm_sb[:, 128:], start=True, stop=True,
                                  skip_group_check=True)
                    wait_once(s_in[chunk_of(in_chunks, t)])
                    if t < n_dense:
                        wait_once(s_atd[chunk_of(atd_chunks, t)])
                        return atd_sb[:, t, :]
                    j = (t - n_dense) // 2
                    if j + 1 > g_thr:
                        te.wait_ge(s_g, j + 1)
                        g_thr = j + 1
                    return at_sb[:, j, (t - n_dense) % 2, :]

                def mm(t, i, rhs, last):
                    m = te.matmul(psums[i][:],
                                  inT_sb[:, t, 128 * i:128 * (i + 1)],
                                  rhs, start=False, stop=last)
                    if last:
                        m.then_inc(s_ps, 1)

                split = NT - TAIL_SPLIT
                for t in range(split):
                    rhs = rhs_of(t)
                    for i in range(NB):
                        mm(t, i, rhs, False)
                # tail: finish batch-half 0 completely so its copy/out-DMA
                # overlap the remaining batch-half-1 matmuls
                tail_rhs = [rhs_of(t) for t in range(split, NT)]
                for i in range(NB):
                    for t in range(split, NT):
                        mm(t, i, tail_rhs[t - split], t == NT - 1)

    nc.compile()
    return nc


def _build_program(wpad: int, n_dense: int):
    key = (wpad, n_dense)
    if key not in _prog_cache:
        _prog_cache[key] = _build_program_raw(wpad, n_dense)
    return _prog_cache[key]


def _prepare(input, condensed_weight, input_mask, bias):
    """Host-side repack: dedupe + CSC-bin the sparse weights, cast/transpose
    the activations. Returns (in_maps, wpad, n_dense)."""
    # input^T bf16 tiled [128f, NT, B]: v[p, t, b] = input[b, 128t + p]
    inT = np.ascontiguousarray(
        input.astype(_BF16).T.reshape(NT, 128, B).transpose(1, 0, 2))

    # dedupe (o, f) pairs, summing weights in f64
    o_idx = np.repeat(np.arange(OUT_F, dtype=np.int64), K)
    f_idx = input_mask.ravel().astype(np.int64)
    w = condensed_weight.ravel()
    key = (o_idx << 12) | f_idx
    uk, inv = np.unique(key, return_inverse=True)
    sums = np.bincount(inv, weights=w.astype(np.float64))
    o_u = (uk >> 12).astype(np.int64)
    f_u = (uk & (IN_F - 1)).astype(np.int64)
    v_u = sums.astype(np.float32)

    core = o_u // O_SH
    o_loc = o_u % O_SH
    t_id = f_u // 128
    p_f = f_u % 128

    n_dense = N_DENSE_TILES
    nt_s = NT - n_dense
    npair = nt_s // 2

    dense_m = t_id < n_dense
    if n_dense:
        atd = np.zeros((N_CORES, 128, n_dense, O_SH), dtype=_BF16)
        atd[core[dense_m], p_f[dense_m], t_id[dense_m], o_loc[dense_m]] = \
            v_u[dense_m]

    wpad = 2
    if npair:
        sm = ~dense_m
        ts = t_id[sm] - n_dense
        s_core, s_p, s_o, s_v = core[sm], p_f[sm], o_loc[sm], v_u[sm]
        s_pair = ts // 2
        # index within the merged pair tile: second tile offset by O_SH
        s_idx = s_o + O_SH * (ts % 2)
        # rank of each entry within its (core, partition, pair) group
        g = (s_core * 128 + s_p) * npair + s_pair
        order = np.argsort(g, kind="stable")
        gs = g[order]
        change = np.r_[True, gs[1:] != gs[:-1]]
        seg_start = np.flatnonzero(change)
        seg_id = np.cumsum(change) - 1
        rank = np.arange(gs.size) - seg_start[seg_id]

        maxc = int(rank.max()) + 1 if gs.size else 0
        wpad = max(2, (maxc + 1) // 2 * 2)

        idx_arr = np.full((N_CORES, 128, npair, wpad), -1, dtype=np.int16)
        val_arr = np.zeros((N_CORES, 128, npair, wpad), dtype=_BF16)
        idx_arr[s_core[order], s_p[order], s_pair[order], rank] = \
            s_idx[order].astype(np.int16)
        val_arr[s_core[order], s_p[order], s_pair[order], rank] = s_v[order]

    in_maps = []
    for c in range(N_CORES):
        m = {
            "inT": inT,
            "bias": np.ascontiguousarray(
                bias[c * O_SH:(c + 1) * O_SH].reshape(1, O_SH)
            ).astype(_BF16),
        }
        if npair:
            m["cscidx"] = np.ascontiguousarray(idx_arr[c])
            m["cscval"] = np.ascontiguousarray(val_arr[c])
        if n_dense:
            m["atd"] = np.ascontiguousarray(atd[c])
        in_maps.append(m)
    return in_maps, wpad, n_dense


def kernel(input, condensed_weight, input_mask, bias,
           _run_kwargs=None, _res_box=None):
    """Full inputs in, full output out. Shards over 8 NeuronCores inside."""
    from concourse.bass_utils import run_bass_kernel_spmd

    in_maps, wpad, n_dense = _prepare(
        np.asarray(input), np.asarray(condensed_weight),
        np.asarray(input_mask), np.asarray(bias))
    nc = _build_program(wpad, n_dense)

    res = run_bass_kernel_spmd(nc, in_maps, list(range(N_CORES)),
                               **(_run_kwargs or {}))
    if _res_box is not None:
        _res_box["results"] = res

    out = np.concatenate(
        [np.asarray(res.results[c]["out"]).reshape(B, O_SH)
         for c in range(N_CORES)], axis=1)
    return out.astype(np.float32)


# revision 11
# speedup vs baseline: 1.0232x; 1.0232x over previous
"""CondensedLinearFineGrained on 8 TRN2 NeuronCores.

Math: out[b,o] = sum_k W[o,k] * input[b, mask[o,k]] + bias[o]
with B=256, IN_F=4096, OUT_F=4096, K=256.

Strategy
--------
Reformulate as a dense matmul:  out = input @ A^T + bias  where
A[o,f] = sum_{k: mask[o,k]==f} W[o,k]  (duplicates within a row are summed).

Sharding: output neurons, 512 per core. Per core:
  - input^T arrives as bf16 f-major tiles [128f x 32t x 256b] (2MB)
  - The first N_DENSE_TILES A^T f-tiles [128f x 512o] bf16 are shipped
    pre-densified from the host; the rest are built on-device by gpsimd
    local_scatter from host-repacked CSC (per-feature (o,weight) lists,
    deduped, -1-padded, int16 indices), two tiles per call.
  - TensorE accumulates psum[128b x 512o] over the 32 f-tiles; PSUM is
    seeded with bias via a K=1 bf16 matmul (ones^T @ bias broadcasts it
    across partitions).

Schedule (all timings vs the ~35.8us baseline this replaces):
  - PE issues N_WARM dependency-free dummy matmuls immediately after the
    engine preamble so the HAM clock gate reaches 2.4GHz before the real
    stream starts (was: warm MMs gated on a vector memset, HAM flipped
    5us INTO the real stream).
  - DMA chunks are small at the head (1-2 tiles) and grow, interleaved
    across the two HWDGE queues in PE-consumption order, so the first
    real matmul starts ~4us earlier.
  - Output is bf16 (error budget allows; host casts back to f32) and the
    last TAIL_SPLIT tiles run batch-half-0 first so psum0's copy/out-DMA
    overlap the remaining matmuls.
  - Semaphores are recycled INSIDE the single block (sync waits on
    done-sems then clears); the separate cleanup block cost ~4us of
    instruction-fetch barrier at the end.
"""

import numpy as np
import ml_dtypes

B = 256
IN_F = 4096
OUT_F = 4096
K = 256
N_CORES = 8
O_SH = OUT_F // N_CORES  # 512 output rows per core
NT = IN_F // 128         # 32 feature tiles
NB = B // 128            # 2 batch tiles

# f-tiles [0, N_DENSE_TILES) are DMA'd pre-densified; the rest are scattered
# on-device by gpsimd, two tiles per local_scatter. NT - N_DENSE_TILES must
# be even.
# All 32 f-tiles ship pre-densified: the gpsimd local_scatter alternative
# saves ~1.75MB of DMA but its ~6us IRAM library load throttles ALL DMA to
# ~half rate for that window (observed in every profiled run), and Pool ends
# up pacing the back half of the stream — a net loss.
N_DENSE_TILES = 32
N_WARM = 12       # dependency-free dummy PE matmuls to release the HAM throttle
TAIL_SPLIT = 4    # trailing tiles run b0-first so psum0 finishes early
OUT_BF16 = True

# progressive chunk sizes (in f-tiles) for the two DMA streams. Chunks below
# ~256KB waste ring time on the per-transfer completion fence (~1.3us), so
# keep them chunky.
IN_CHUNKS = [4, 4, 8, 8, 4, 4]
ATD_CHUNKS = [2, 2, 4, 4, 4, 4, 4, 4, 2, 2]
# tiles at which an extra dummy matmul is inserted before the chunk waits:
# keeps the HAM clock gate warm across DMA-paced stalls in the dense phase
# (free while the PE is DMA-paced; excluded from the PE-bound tail)
WARM_AT = (2, 4, 6, 8, 10, 12, 14, 16, 20)

_BF16 = ml_dtypes.bfloat16

_prog_cache = {}


def _chunk_bounds(total, sizes):
    out, p, i = [], 0, 0
    while p < total:
        s = min(sizes[min(i, len(sizes) - 1)], total - p)
        out.append((p, p + s))
        p += s
        i += 1
    return out


def _build_program_raw(wpad: int, n_dense: int):
    """Hand-scheduled SPMD program: explicit per-engine streams + semaphores."""
    from contextlib import ExitStack
    from concourse import bacc, mybir, library_config

    nt_s = NT - n_dense
    npair = nt_s // 2
    assert nt_s % 2 == 0

    atd_chunks = _chunk_bounds(n_dense, ATD_CHUNKS)
    in_chunks = _chunk_bounds(NT, IN_CHUNKS)
    n_atd_ch = len(atd_chunks)
    n_in_ch = len(in_chunks)

    def chunk_of(chunks, t):
        for c, (c0, c1) in enumerate(chunks):
            if c0 <= t < c1:
                return c
        raise AssertionError

    nc = bacc.Bacc("TRN2", target_bir_lowering=False, debug=False)
    dt = mybir.dt
    out_dt = dt.bfloat16 if OUT_BF16 else dt.float32

    inT_d = nc.dram_tensor("inT", [128, NT, B], dt.bfloat16, kind="ExternalInput")
    bias_d = nc.dram_tensor("bias", [1, O_SH], dt.bfloat16, kind="ExternalInput")
    if npair:
        idx_d = nc.dram_tensor("cscidx", [128, npair, wpad], dt.int16,
                               kind="ExternalInput")
        val_d = nc.dram_tensor("cscval", [128, npair, wpad], dt.bfloat16,
                               kind="ExternalInput")
    if n_dense:
        atd_d = nc.dram_tensor("atd", [128, n_dense, O_SH], dt.bfloat16,
                               kind="ExternalInput")
    out_d = nc.dram_tensor("out", [NB, 128, O_SH], out_dt,
                           kind="ExternalOutput")

    inT_sb = nc.alloc_sbuf_tensor("inT_sb", [128, NT, B], dt.bfloat16).ap()
    bias_sb = nc.alloc_sbuf_tensor("bias_sb", [1, O_SH], dt.bfloat16).ap()
    ones_sb = nc.alloc_sbuf_tensor("ones_sb", [1, 128], dt.bfloat16).ap()
    # warm region is intentionally never initialized: the dummy matmuls only
    # exist to keep the PE busy, their values are irrelevant
    warm_sb = nc.alloc_sbuf_tensor("warm_sb", [128, 128 + O_SH],
                                   dt.bfloat16).ap()
    if npair:
        idx_sb = nc.alloc_sbuf_tensor("idx_sb", [128, npair, wpad],
                                      dt.int16).ap()
        val_sb = nc.alloc_sbuf_tensor("val_sb", [128, npair, wpad],
                                      dt.bfloat16).ap()
        at_sb = nc.alloc_sbuf_tensor("at_sb", [128, npair, 2, O_SH],
                                     dt.bfloat16).ap()
    if n_dense:
        atd_sb = nc.alloc_sbuf_tensor("atd_sb", [128, n_dense, O_SH],
                                      dt.bfloat16).ap()
    outs_sb = [nc.alloc_sbuf_tensor(f"out_sb{i}", [128, O_SH], out_dt).ap()
               for i in range(NB)]

    psums = [nc.alloc_psum_tensor(f"ps{i}", [128, O_SH], dt.float32).ap()
             for i in range(NB)]
    ps_warm = nc.alloc_psum_tensor("ps_warm", [128, O_SH], dt.float32).ap()

    with ExitStack() as ctx:
        sem = lambda name: ctx.enter_context(nc.semaphore(name))
        # One semaphore per DMA: sub-transfers of back-to-back DMAs on one
        # queue can complete out of order, so prefix thresholds on a shared
        # semaphore would be unsound.
        s_bias = sem("s_bias")
        s_ci = sem("s_ci") if npair else None
        s_cv = sem("s_cv") if npair else None
        s_in = [sem(f"s_in{c}") for c in range(n_in_ch)]
        s_atd = [sem(f"s_atd{c}") for c in range(n_atd_ch)]
        # out-DMA completion sems: incremented (the BIR verifier requires a
        # sem update on every DMA) but never waited or cleared — the
        # runtime's queue drain covers output completion.
        s_od = [sem(f"s_od{i}") for i in range(NB)]
        s_g = sem("s_g")    # scatter pairs published
        s_v = sem("s_v")    # DVE consts ready
        s_ps = sem("s_ps")  # PE accumulation done per psum
        s_cp = sem("s_cp")  # psum->sbuf copies done

        recycle_sems = ([s_bias] + ([s_ci, s_cv] if npair else [])
                        + s_in + s_atd + [s_g, s_v, s_ps, s_cp])

        # Interleave both input streams across the two HWDGE queues in
        # PE-consumption order (PE eats 2 bytes of dense A^T per byte of
        # input^T), greedily byte-balanced so neither queue starves the PE.
        # The CSC arrays ride the same queues (the gpsimd SWDGE path is
        # blocked ~6us by the local_scatter IRAM library load), placed so
        # they land by the time the library is ready (~13.4us).
        feed = []
        for c, (c0, c1) in enumerate(atd_chunks):
            feed.append((c0, 0, atd_sb[:, c0:c1, :], atd_d[:, c0:c1, :],
                         s_atd[c], (c1 - c0) * O_SH * 2))
        for c, (c0, c1) in enumerate(in_chunks):
            feed.append((c0, 1, inT_sb[:, c0:c1, :], inT_d[:, c0:c1, :],
                         s_in[c], (c1 - c0) * B * 2))
        if npair:
            # CSC leads both queues: the scatters can't start before the
            # ~6us local_scatter IRAM load finishes (~13.4us), and Pool
            # paces the whole back half if its input arrives later
            csc_b = 128 * npair * wpad * 2
            feed.append((-1, 2, idx_sb[:], idx_d[:], s_ci, csc_b))
            feed.append((-1, 3, val_sb[:], val_d[:], s_cv, csc_b))
        feed.sort(key=lambda f: (f[0], f[1]))
        qa, qb, ba, bb = [], [], 0, 0
        for _, _, dst, src, s, w in feed:
            if ba <= bb:
                qa.append((dst, src, s)); ba += w
            else:
                qb.append((dst, src, s)); bb += w

        with nc.Block() as block:

            @block.sync
            def _(sy):
                for dst, src, s in qa:
                    sy.dma_start(out=dst, in_=src).then_inc(s, 16)
                # out1 DMA (out0 goes on scalar); completion not waited —
                # the runtime queue drain covers it
                sy.wait_ge(s_cp, 2)
                sy.dma_start(out=out_d[1],
                             in_=outs_sb[1][:]).then_inc(s_od[1], 16)
                # recycle semaphores inline: all waiters are done once the
                # copies (s_cp) and scatters (s_g) have completed, so the
                # next execution of this NEFF starts from zero without a
                # separate cleanup block (which would pay its own
                # instruction-fetch barrier).
                if npair:
                    sy.wait_ge(s_g, npair)
                for s in recycle_sems:
                    sy.sem_clear(s)

            @block.scalar
            def _(sc):
                sc.dma_start(out=bias_sb[:], in_=bias_d[:]).then_inc(s_bias, 16)
                for dst, src, s in qb:
                    sc.dma_start(out=dst, in_=src).then_inc(s, 16)
                sc.wait_ge(s_cp, 1)
                sc.dma_start(out=out_d[0],
                             in_=outs_sb[0][:]).then_inc(s_od[0], 16)

            @block.vector
            def _(v):
                v.memset(ones_sb[:], 1.0)
                v.drain()
                v.sem_inc(s_v, 1)
                for i in range(NB):
                    v.wait_ge(s_ps, i + 1)
                    v.tensor_copy(outs_sb[i][:],
                                  psums[i][:]).then_inc(s_cp, 1)

            if npair:
                @block.gpsimd
                def _(g):
                    g.load_library(library_config.local_scatter)
                    g.wait_ge(s_ci, 16)
                    g.wait_ge(s_cv, 16)
                    for j in range(npair):
                        g.local_scatter(
                            at_sb[:, j],
                            val_sb[:, j],
                            idx_sb[:, j],
                            channels=128,
                            num_elems=2 * O_SH,
                            num_idxs=wpad,
                        ).then_inc(s_g, 1)

            @block.tensor
            def _(te):
                # dependency-free dummy matmuls from t~0: the HAM clock gate
                # needs ~3.4us of sustained PE activity to reach 2.4GHz, so
                # these run while the input DMAs are still in flight. The
                # warm region is uninitialized garbage; results go to a PSUM
                # bank that is never read.
                for _ in range(N_WARM):
                    te.matmul(ps_warm[:], warm_sb[:, :128], warm_sb[:, 128:],
                              start=True, stop=True, skip_group_check=True)
                te.wait_ge(s_v, 1)
                te.wait_ge(s_bias, 16)
                for i in range(NB):
                    te.matmul(psums[i][:], ones_sb[:], bias_sb[:],
                              start=True, stop=False)

                seen = set()
                g_thr = 0

                def wait_once(s):
                    if s.name not in seen:
                        te.wait_ge(s, 16)
                        seen.add(s.name)

                def rhs_of(t):
                    nonlocal g_thr
                    if t in WARM_AT:
                        # insurance against HAM re-throttle: a dummy matmul
                        # right before a chunk wait keeps the PE from sitting
                        # fully idle through a DMA-paced stall
                        te.matmul(ps_warm[:], warm_sb[:, :128],
                                  warm_sb[:, 128:], start=True, stop=True,
                                  skip_group_check=True)
                    wait_once(s_in[chunk_of(in_chunks, t)])
                    if t < n_dense:
                        wait_once(s_atd[chunk_of(atd_chunks, t)])
                        return atd_sb[:, t, :]
                    j = (t - n_dense) // 2
                    if j + 1 > g_thr:
                        te.wait_ge(s_g, j + 1)
                        g_thr = j + 1
                    return at_sb[:, j, (t - n_dense) % 2, :]

                def mm(t, i, rhs, last):
                    m = te.matmul(psums[i][:],
                                  inT_sb[:, t, 128 * i:128 * (i + 1)],
                                  rhs, start=False, stop=last)
                    if last:
                        m.then_inc(s_ps, 1)

                split = NT - TAIL_SPLIT
                for t in range(split):
                    rhs = rhs_of(t)
                    for i in range(NB):
                        mm(t, i, rhs, False)
                # tail: finish batch-half 0 completely so its copy/out-DMA
                # overlap the remaining batch-half-1 matmuls
                tail_rhs = [rhs_of(t) for t in range(split, NT)]
                for i in range(NB):
                    for t in range(split, NT):
                        mm(t, i, tail_rhs[t - split], t == NT - 1)

    nc.compile()
    return nc


def _build_program(wpad: int, n_dense: int):
    key = (wpad, n_dense)
    if key not in _prog_cache:
        _prog_cache[key] = _build_program_raw(wpad, n_dense)
    return _prog_cache[key]


def _prepare(input, condensed_weight, input_mask, bias):
    """Host-side repack: dedupe + CSC-bin the sparse weights, cast/transpose
    the activations. Returns (in_maps, wpad, n_dense)."""
    # input^T bf16 tiled [128f, NT, B]: v[p, t, b] = input[b, 128t + p]
    inT = np.ascontiguousarray(
        input.astype(_BF16).T.reshape(NT, 128, B).transpose(1, 0, 2))

    # dedupe (o, f) pairs, summing weights in f64
    o_idx = np.repeat(np.arange(OUT_F, dtype=np.int64), K)
    f_idx = input_mask.ravel().astype(np.int64)
    w = condensed_weight.ravel()
    key = (o_idx << 12) | f_idx
    uk, inv = np.unique(key, return_inverse=True)
    sums = np.bincount(inv, weights=w.astype(np.float64))
    o_u = (uk >> 12).astype(np.int64)
    f_u = (uk & (IN_F - 1)).astype(np.int64)
    v_u = sums.astype(np.float32)

    core = o_u // O_SH
    o_loc = o_u % O_SH
    t_id = f_u // 128
    p_f = f_u % 128

    n_dense = N_DENSE_TILES
    nt_s = NT - n_dense
    npair = nt_s // 2

    dense_m = t_id < n_dense
    if n_dense:
        atd = np.zeros((N_CORES, 128, n_dense, O_SH), dtype=_BF16)
        atd[core[dense_m], p_f[dense_m], t_id[dense_m], o_loc[dense_m]] = \
            v_u[dense_m]

    wpad = 2
    if npair:
        sm = ~dense_m
        ts = t_id[sm] - n_dense
        s_core, s_p, s_o, s_v = core[sm], p_f[sm], o_loc[sm], v_u[sm]
        s_pair = ts // 2
        # index within the merged pair tile: second tile offset by O_SH
        s_idx = s_o + O_SH * (ts % 2)
        # rank of each entry within its (core, partition, pair) group
        g = (s_core * 128 + s_p) * npair + s_pair
        order = np.argsort(g, kind="stable")
        gs = g[order]
        change = np.r_[True, gs[1:] != gs[:-1]]
        seg_start = np.flatnonzero(change)
        seg_id = np.cumsum(change) - 1
        rank = np.arange(gs.size) - seg_start[seg_id]

        maxc = int(rank.max()) + 1 if gs.size else 0
        wpad = max(2, (maxc + 1) // 2 * 2)

        idx_arr = np.full((N_CORES, 128, npair, wpad), -1, dtype=np.int16)
        val_arr = np.zeros((N_CORES, 128, npair, wpad), dtype=_BF16)
        idx_arr[s_core[order], s_p[order], s_pair[order], rank] = \
            s_idx[order].astype(np.int16)
        val_arr[s_core[order], s_p[order], s_pair[order], rank] = s_v[order]

    in_maps = []
    for c in range(N_CORES):
        m = {
            "inT": inT,
            "bias": np.ascontiguousarray(
                bias[c * O_SH:(c + 1) * O_SH].reshape(1, O_SH)
            ).astype(_BF16),
        }
        if npair:
            m["cscidx"] = np.ascontiguousarray(idx_arr[c])
            m["cscval"] = np.ascontiguousarray(val_arr[c])
        if n_dense:
            m["atd"] = np.ascontiguousarray(atd[c])
        in_maps.append(m)
    return in_maps, wpad, n_dense


def kernel(input, condensed_weight, input_mask, bias,
           _run_kwargs=None, _res_box=None):
    """Full inputs in, full output out. Shards over 8 NeuronCores inside."""
    from concourse.bass_utils import run_bass_kernel_spmd

    in_maps, wpad, n_dense = _prepare(
        np.asarray(input), np.asarray(condensed_weight),
        np.asarray(input_mask), np.asarray(bias))
    nc = _build_program(wpad, n_dense)

    res = run_bass_kernel_spmd(nc, in_maps, list(range(N_CORES)),
                               **(_run_kwargs or {}))
    if _res_box is not None:
        _res_box["results"] = res

    out = np.concatenate(
        [np.asarray(res.results[c]["out"]).reshape(B, O_SH)
         for c in range(N_CORES)], axis=1)
    return out.astype(np.float32)
